# revision 24
# baseline (speedup 1.0000x reference)
"""Trainium2 Bass kernel for ChannelCompression:
   y = minmax_norm_spatial(leaky_relu(circulant_1x1_conv(x) + b))

Sharding: pure data parallel over batch (16 batches -> 2 per core x 8 cores).

Per-core strategy (memory-roofline bound: read x once, write y once):
  - View each batch as [C=16, G=8, S=32768] and stack (c,g) onto the 128
    SBUF partitions.  The circulant 16x16 conv becomes one 128x128
    block-structured matmul weight kron(W2.T, I8), so every PE column
    computes all 16 output channels for 8 spatial groups at once.
  - x is DMA-loaded with an inline f32->bf16 cast (SWDGE), so the matmul
    runs in bf16 (4x PE rate vs fp32) and the resident y buffer is bf16
    (half SBUF, 2x DVE reduce rate).  Precision budget: harness gate is
    rel 2e-2; bf16 rounding contributes ~1e-3.
  - Pass 1 streams x tiles in, matmuls into PSUM (fp32), applies
    leaky-relu (+bias) on ScalarE while writing bf16 into the resident y
    buffer, and reduces per-partition min/max on DVE.
  - Per-batch stats are folded across the 8 spatial groups via tiny PE
    transposes into free-dim space, reduced, inverted, and broadcast back
    to per-partition scale/bias with two tiny selector matmuls.
  - Pass 2 normalizes the resident y on GpSimd (bf16 in, f32 out) and
    streams the result out via HWDGE on SyncE.  Engine assignment keeps
    every lane under the DMA pace: loads emit on GpSimd (SWDGE, needed
    for the cast), stores on SyncE, activations on ScalarE, reduces on
    DVE, normalize on GpSimd.
  - Software pipeline: pass2 of batch b interleaves with pass1 of batch
    b+1; a deep prefetch window (PRE) keeps the DMA queues fed across the
    per-batch stats fold.
"""

import numpy as np
from contextlib import ExitStack

import concourse.bacc as bacc
import concourse.tile as tile
import concourse.bass as bass
from concourse import mybir
from concourse.bass_utils import run_bass_kernel_spmd

F32 = mybir.dt.float32
BF16 = mybir.dt.bfloat16
AF = mybir.ActivationFunctionType
ALU = mybir.AluOpType
AX = mybir.AxisListType

N_CORES = 8
B, C, H, W = 16, 16, 512, 512
G = 8                   # spatial groups stacked into partitions
BP = B // N_CORES       # batches per core
S_FULL = (H * W) // G   # 32768 spatial elems per group
TS = 2048               # columns per resident y tile
PT = 1024               # columns per PSUM tile (2 banks)
MM = 512                # columns per matmul (1 PSUM bank, fp32 PSUM)
EPS = 1e-8
NEG_SLOPE = 0.1
PRE = 5                 # next-batch pass1 tiles pre-issued before the fold


def build_nc(S=S_FULL, ts=TS):
    n_t = S // ts
    nc = bacc.Bacc("TRN2", target_bir_lowering=False)

    xs = nc.dram_tensor("x", [BP, C, G, S], F32, kind="ExternalInput")
    wbd = nc.dram_tensor("wbd", [128, 128], BF16, kind="ExternalInput")
    ident = nc.dram_tensor("ident", [128, 128], F32, kind="ExternalInput")
    sel = nc.dram_tensor("sel", [32, 2, 128], F32, kind="ExternalInput")
    bb = nc.dram_tensor("b128", [128, 1], F32, kind="ExternalInput")
    ys = nc.dram_tensor("y", [BP, C, G, S], F32, kind="ExternalOutput")

    with tile.TileContext(nc) as tc, ExitStack() as ctx:
        consts = ctx.enter_context(tc.tile_pool(name="consts", bufs=1))
        # xpool deep enough that ALL next-batch load emissions clear the Q7
        # queue before the first store is emitted — descriptors enter the
        # SWDGE ring in emission order, and a late load queued behind
        # stores re-mixes the HBM read/write phases.
        xpool = ctx.enter_context(tc.tile_pool(name="xpool", bufs=12))
        ypool = ctx.enter_context(tc.tile_pool(name="ypool", bufs=n_t + PRE + 1))
        opool = ctx.enter_context(tc.tile_pool(name="opool", bufs=8))
        spool = ctx.enter_context(tc.tile_pool(name="stats", bufs=1))
        small = ctx.enter_context(tc.tile_pool(name="small", bufs=2))
        psum = ctx.enter_context(tc.tile_pool(name="psum", bufs=3, space="PSUM"))
        psmall = ctx.enter_context(tc.tile_pool(name="psmall", bufs=2, space="PSUM"))

        # All consts go over HWDGE (sync) so the Q7/SWDGE queue is free for
        # x-tile emission from the first instruction (wbd is pre-cast to
        # bf16 on the host).
        wbd_sb = consts.tile([128, 128], BF16)
        nc.sync.dma_start(out=wbd_sb, in_=wbd[:])
        id_sb = consts.tile([128, 128], F32)
        nc.sync.dma_start(out=id_sb, in_=ident[:])
        sel_sb = consts.tile([32, 2, 128], F32)
        nc.sync.dma_start(out=sel_sb, in_=sel[:])
        b_sb = consts.tile([128, 1], F32)
        nc.sync.dma_start(out=b_sb, in_=bb[:])
        # Tiny warm-up Prelu: forces the one-time ACT_TABLE_LOAD (~1.3us)
        # to happen during the DMA ramp instead of before the first real
        # activation.
        warm = consts.tile([128, 1], F32)
        nc.scalar.activation(
            out=warm, in_=b_sb, func=AF.Prelu, bias=b_sb, scale=1.0,
            alpha=NEG_SLOPE,
        )

        state = {}

        # pass 1 is split into three separately-issuable pieces so each
        # engine's (in-order) instruction queue can be sequenced to never
        # wait behind an op with slower dependencies:
        #   pass1_load   -> GpSimd Q7 emission + SWDGE ring
        #   pass1_mmact  -> PE matmuls + ScalarE Prelu (PSUM->SBUF bf16)
        #   pass1_stat   -> DVE running min/max chains
        def pass1_load(bi, i):
            xt = xpool.tile([128, ts], BF16, tag="x")
            nc.gpsimd.dma_start(out=xt, in_=xs[bi, :, :, i * ts:(i + 1) * ts])
            return xt

        def pass1_mmact(bi, i, xt):
            y_tiles = state[bi][4]
            yt = ypool.tile([128, ts], BF16, tag="y")
            for j in range(ts // PT):
                pt = psum.tile([128, PT], F32, tag="ps")
                for k in range(PT // MM):
                    c0 = k * MM
                    nc.tensor.matmul(
                        pt[:, c0:c0 + MM],
                        wbd_sb,
                        xt[:, j * PT + c0:j * PT + c0 + MM],
                        start=True,
                        stop=True,
                    )
                # y = leaky_relu(conv + b): fused PSUM->SBUF on ScalarE
                nc.scalar.activation(
                    out=yt[:, j * PT:(j + 1) * PT],
                    in_=pt,
                    func=AF.Prelu,
                    bias=b_sb,
                    scale=1.0,
                    alpha=NEG_SLOPE,
                )
            y_tiles.append(yt)

        def pass1_stat(bi, i):
            """Running elementwise min/max via tensor_tensor (bf16 2x_1P,
            ~1.1us) instead of per-tile tensor_reduce (1x, ~2us) — and two
            interleaved chains (even/odd tiles) so the serial RAW dependency
            is never rate-limiting."""
            rminA, rminB, rmaxA, rmaxB, y_tiles = state[bi][:5]
            rmin = rminA if i % 2 == 0 else rminB
            rmax = rmaxA if i % 2 == 0 else rmaxB
            yt = y_tiles[i]
            if i < 2:
                # chain init: tensor_copy runs 4x (0.53us) vs TT's 2x (1.1us)
                nc.vector.tensor_copy(rmin, yt)
                nc.vector.tensor_copy(rmax, yt)
            else:
                nc.vector.tensor_tensor(out=rmin, in0=rmin, in1=yt, op=ALU.min)
                nc.vector.tensor_tensor(out=rmax, in0=rmax, in1=yt, op=ALU.max)

        def pass1_tile(bi, i):
            xt = pass1_load(bi, i)
            pass1_mmact(bi, i, xt)
            pass1_stat(bi, i)

        def stats_fold(bi):
            """Fold per-partition stats into per-partition scale/bias [128,2].

            All small ops live on DVE + PE (fold copies via DVE tensor_copy)
            so the Scalar queue stays free for activations/normalizes and
            GpSimd stays free for SWDGE load emission.
            """
            rminA, rminB, rmaxA, rmaxB = state[bi][:4]
            nc.vector.tensor_tensor(out=rminA, in0=rminA, in1=rminB, op=ALU.min)
            nc.vector.tensor_tensor(out=rmaxA, in0=rmaxA, in1=rmaxB, op=ALU.max)
            s2 = small.tile([128, 2], F32, tag="s2")
            nc.vector.tensor_reduce(out=s2[:, 0:1], in_=rminA, axis=AX.X, op=ALU.min)
            nc.vector.tensor_reduce(out=s2[:, 1:2], in_=rmaxA, axis=AX.X, op=ALU.max)
            # transpose [128,1] stats into free dim (partition 0)
            ptr_min = psmall.tile([1, 128], F32, tag="psm")
            nc.tensor.transpose(ptr_min, s2[:, 0:1], id_sb)
            ptr_max = psmall.tile([1, 128], F32, tag="psm")
            nc.tensor.transpose(ptr_max, s2[:, 1:2], id_sb)
            tl = small.tile([1, 256], F32, tag="tl")
            nc.vector.tensor_copy(tl[:, 0:128], ptr_min)
            nc.vector.tensor_copy(tl[:, 128:256], ptr_max)
            # reduce over the 8 groups (free index p = o*8+g)
            u = small.tile([1, 32], F32, tag="u")
            nc.vector.tensor_reduce(
                out=u[:, 0:16],
                in_=tl[:, 0:128].rearrange("p (o g) -> p o g", g=G),
                axis=AX.X,
                op=ALU.min,
            )
            nc.vector.tensor_reduce(
                out=u[:, 16:32],
                in_=tl[:, 128:256].rearrange("p (o g) -> p o g", g=G),
                axis=AX.X,
                op=ALU.max,
            )
            # scale = 1/(mx-mn+eps); nbias = -mn*scale
            v = small.tile([1, 16], F32, tag="v")
            nc.vector.tensor_sub(out=v, in0=u[:, 16:32], in1=u[:, 0:16])
            vv = small.tile([1, 16], F32, tag="vv")
            nc.vector.tensor_scalar(
                out=vv, in0=v, scalar1=EPS, scalar2=None, op0=ALU.add
            )
            pk = small.tile([1, 32], F32, tag="pk")
            nc.vector.reciprocal(out=pk[:, 0:16], in_=vv)
            tmp = small.tile([1, 16], F32, tag="tmp")
            nc.vector.tensor_mul(out=tmp, in0=u[:, 0:16], in1=pk[:, 0:16])
            nc.vector.tensor_scalar(
                out=pk[:, 16:32], in0=tmp, scalar1=-1.0, scalar2=None, op0=ALU.mult
            )
            # broadcast [1,32] free-dim -> per-partition [128,2] via transpose
            # + selector matmuls (sel[k,0,p]=d(k==p//8), sel[k,1,p]=d(k-16==p//8))
            pz = psmall.tile([32, 1], F32, tag="psm")
            nc.tensor.transpose(pz, pk, id_sb[0:1, 0:1])
            zs = small.tile([32, 1], F32, tag="zs")
            nc.vector.tensor_copy(zs, pz)
            pb1 = psmall.tile([128, 1], F32, tag="psm")
            nc.tensor.matmul(pb1, sel_sb[:, 0, :], zs, start=True, stop=True)
            pb2 = psmall.tile([128, 1], F32, tag="psm")
            nc.tensor.matmul(pb2, sel_sb[:, 1, :], zs, start=True, stop=True)
            sc = small.tile([128, 2], F32, tag="sc")
            nc.vector.tensor_copy(sc[:, 0:1], pb1)
            nc.vector.tensor_copy(sc[:, 1:2], pb2)
            return sc

        def pass2_tile(bi, i, sc, on_scalar=False):
            """Normalize resident y tile and stream out.

            Normalize runs in bf16 on DVE (tensor_scalar, sub-us) or on
            ScalarE (activation Identity) — the caller picks whichever
            engine is off the critical path — and the store is an SWDGE
            cast-DMA (bf16 SBUF -> f32 HBM).  GpSimd compute is never used
            here: it would contend with the DVE min/max chains for the
            shared SBUF port pair.  The bf16 output rounding adds ~1e-3 to
            the (2e-2-gated) error.
            """
            y_tiles = state[bi][4]
            ot = opool.tile([128, ts], BF16, tag="o")
            if on_scalar:
                nc.scalar.activation(
                    out=ot,
                    in_=y_tiles[i],
                    func=AF.Identity,
                    bias=sc[:, 1:2],
                    scale=sc[:, 0:1],
                )
            else:
                nc.vector.tensor_scalar(
                    out=ot,
                    in0=y_tiles[i],
                    scalar1=sc[:, 0:1],
                    scalar2=sc[:, 1:2],
                    op0=ALU.mult,
                    op1=ALU.add,
                )
            nc.gpsimd.dma_start(out=ys[bi, :, :, i * ts:(i + 1) * ts], in_=ot)

        # software pipeline: pass1(0); then per batch: pre-emit the first
        # next-batch tiles (keeps the DMA queues busy through the stats
        # fold), fold stats, then interleave the rest of pass1(bi+1) with
        # pass2(bi) (pass1 first so loads stay ahead of stores).
        for bi in range(BP):
            state[bi] = (
                spool.tile([128, ts], BF16, tag="rminA", name=f"rminA{bi}"),
                spool.tile([128, ts], BF16, tag="rminB", name=f"rminB{bi}"),
                spool.tile([128, ts], BF16, tag="rmaxA", name=f"rmaxA{bi}"),
                spool.tile([128, ts], BF16, tag="rmaxB", name=f"rmaxB{bi}"),
                [],
            )
        # Software pipeline.  Engines execute their streams strictly in
        # order; the issue order below shapes each queue:
        #   ring/Q7: ALL loads first (b0 then b1), THEN all stores — a pure
        #         HBM-read phase followed by a pure HBM-write phase.  A 1:1
        #         load/store interleave measured ~14% slower per transfer
        #         (read<->write turnaround), raising per-engine DMA work
        #         from ~160us to ~186us.
        #   DVE:  b0 chain -> fold(b0) -> {norm(b0,i); stat(b1,i)} ->
        #         fold(b1) -> norms(b1).  stat(b1,i) trails its load by PRE
        #         tiles so it never stalls the queue.
        #   PE/Scalar: b1 matmuls+acts issued as loads land; only the first
        #         PRE of them run before fold(b0)'s PE transposes so the
        #         fold isn't queued behind the whole b1 batch.
        for i in range(n_t):
            pass1_tile(0, i)
        for bi in range(BP):
            nxt = bi + 1
            xts = {}
            if nxt < BP:
                for j in range(n_t):
                    xts[j] = pass1_load(nxt, j)
                for j in range(PRE):
                    pass1_mmact(nxt, j, xts[j])
            sc = stats_fold(bi)
            # Normalize-engine split: with a next batch in flight, late
            # normalizes (i >= 9) go to ScalarE — idle once the next
            # batch's activations finish — so DVE completes the next chain
            # ~12us sooner and fold(nxt) overlaps the store drain instead
            # of going ring-dry.  The last batch alternates engines so the
            # tail stores are never waiting on a just-in-time normalize.
            for i in range(n_t):
                if nxt < BP:
                    on_scalar = i >= 9
                else:
                    on_scalar = i % 2 == 1
                pass2_tile(bi, i, sc, on_scalar=on_scalar)
                if nxt < BP:
                    if i + PRE < n_t:
                        pass1_mmact(nxt, i + PRE, xts[i + PRE])
                    pass1_stat(nxt, i)

    nc.compile()
    return nc


def host_consts(w, b):
    """Host-side tiny constant tensors fed to every core."""
    import ml_dtypes

    w = np.asarray(w, np.float32).reshape(16)
    b = np.asarray(b, np.float32).reshape(1)
    W2 = np.stack([np.roll(w, o) for o in range(16)], axis=0)   # [O,C]
    wbd = np.kron(W2.T.copy(), np.eye(G, dtype=np.float32))     # [128,128]
    wbd = np.ascontiguousarray(wbd).astype(ml_dtypes.bfloat16)
    ident = np.eye(128, dtype=np.float32)
    sel = np.zeros((32, 2, 128), np.float32)
    for p in range(128):
        sel[p // G, 0, p] = 1.0
        sel[16 + p // G, 1, p] = 1.0
    b128 = np.full((128, 1), float(b[0]), np.float32)
    return wbd, ident, sel, b128


_NC = None
LAST_RESULTS = None


def kernel(x, w, b):
    global _NC, LAST_RESULTS
    x = np.ascontiguousarray(np.asarray(x, np.float32))
    assert x.shape == (B, C, H, W)
    if _NC is None:
        _NC = build_nc()
    wbd, ident, sel, b128 = host_consts(w, b)

    xg = x.reshape(N_CORES, BP, C, G, S_FULL)
    in_maps = [
        {
            "x": np.ascontiguousarray(xg[ci]),
            "wbd": wbd,
            "ident": ident,
            "sel": sel,
            "b128": b128,
        }
        for ci in range(N_CORES)
    ]
    res = run_bass_kernel_spmd(_NC, in_maps, core_ids=list(range(N_CORES)))
    LAST_RESULTS = res
    out = np.concatenate([r["y"].reshape(BP, C, H, W) for r in res.results], axis=0)
    return out
